# revision 18
# baseline (speedup 1.0000x reference)
"""Trainium2 Bass kernel: CodeEncoder attention pooling, vocab-sharded
histogram form with a single cross-core ReduceScatter.

Math per bag: out = sum_c softmax(score(idx_c))_c * table[idx_c]. Scores
depend only on the vocab id (score = W2 tanh(W1 e + b1); b2 cancels in
softmax), so with per-bag vocab counts Cnt[v, bag] (host-built):

    g(v) = exp(score_v)                    (device, score-table MLP)
    num  = (g*table)^T @ Cnt  [bags, 257]  (dense matmul, ones col -> Z)
    out  = num / Z

Sharding: VOCAB-sharded. Core k owns vocab slice [2560k, 2560k+2560):
it runs the score MLP on its slice only (1/8 the table traffic and MLP
flops of the batch-parallel form) and accumulates bf16 partial num/Z
for ALL 3200 bags over its slice, K-contiguous per 128-bag tile so the
PE never idles. One ReduceScatter returns each core its own 400 bags,
which it normalizes and stores. (The collective subsystem here has a
~65us fixed warmup that overlaps compute + ~25us/MB marginal cost, so
exactly one RS is used.)

Length-0 bags (softmax fully masked -> uniform mean of all 64 codes)
do not fit the g-weighted form; they are pooled core-locally by a
small mid-loop dma_gather of their 64*32 rows + one block-diagonal
mean matmul into output rows 400..431, no collective involved. The
host maps those rows back.
"""

import sys

if "/opt/trn_rl_repo" not in sys.path:
    sys.path.insert(0, "/opt/trn_rl_repo")

from contextlib import ExitStack

import numpy as np

B, V, C = 64, 50, 64
NUM_CODE, D, H = 20000, 256, 128
NCORES = 8
BPC = B // NCORES          # batches per core
BAGS = BPC * V             # 400 bags owned per core
GBAGS = B * V              # 3200 global bags
VP = 20480                 # padded vocab
VSL = VP // NCORES         # 2560 vocab per core
NCH = VSL // 128           # 20 vocab chunks per core
NSL = 512                  # score-MLP slice (one f32 psum bank)
NW = D + 2                 # rhs width: 256 emb + ones col + pad
NT = GBAGS // 128          # 25 bag tiles
ESLOT = 32                 # len-0 slots per core
ECODES = ESLOT * C         # 2048 gathered rows for the epilogue
ORT = BAGS + ESLOT         # 432 output rows

_cache = {}


def _build_program():
    import concourse.bass as bass  # noqa: F401
    import concourse.tile as tile
    from concourse import bacc, mybir

    f16 = mybir.dt.float16
    f32 = mybir.dt.float32
    bf16 = mybir.dt.bfloat16
    f8 = mybir.dt.float8e4
    i16 = mybir.dt.int16

    nc = bacc.Bacc("TRN2", target_bir_lowering=False, debug=False,
                   num_devices=NCORES)

    tabt_d = nc.dram_tensor("tabt", [128, 2 * VSL], f16, kind="ExternalInput")
    rhsc_d = nc.dram_tensor("rhsc", [128, NCH * NW], f16, kind="ExternalInput")
    cnt_d = nc.dram_tensor("cnt", [128, NT * NCH * 128], f8,
                           kind="ExternalInput")
    w1t_d = nc.dram_tensor("w1t", [D, H], f16, kind="ExternalInput")
    w2c_d = nc.dram_tensor("w2c", [H, 1], f16, kind="ExternalInput")
    b1_d = nc.dram_tensor("b1", [H, 1], f32, kind="ExternalInput")
    table_d = nc.dram_tensor("table", [NUM_CODE, D], f16, kind="ExternalInput")
    epool_d = nc.dram_tensor("epool", [128, 16 * 32], f16, kind="ExternalInput")
    egidx_d = nc.dram_tensor("egidx", [128, ECODES // 16], i16,
                             kind="ExternalInput")
    out_d = nc.dram_tensor("out", [ORT, D], f32, kind="ExternalOutput")

    groups = [list(range(NCORES))]

    with tile.TileContext(nc) as tc, ExitStack() as ctx:
        const = ctx.enter_context(tc.tile_pool(name="const", bufs=1))
        cntp = ctx.enter_context(tc.tile_pool(name="cntp", bufs=6))
        hp = ctx.enter_context(tc.tile_pool(name="hp", bufs=2))
        obp = ctx.enter_context(tc.tile_pool(name="obp", bufs=3))
        normp = ctx.enter_context(tc.tile_pool(name="normp", bufs=1))
        php = ctx.enter_context(tc.tile_pool(name="ph", bufs=2, space="PSUM"))
        gpp = ctx.enter_context(tc.tile_pool(name="gp", bufs=1, space="PSUM"))
        psp = ctx.enter_context(tc.tile_pool(name="ps", bufs=2, space="PSUM"))
        epp = ctx.enter_context(tc.tile_pool(name="epp", bufs=1, space="PSUM"))
        dram = ctx.enter_context(tc.tile_pool(name="dram", bufs=1,
                                              space="DRAM"))

        part_t = dram.tile([GBAGS, NW], bf16)
        rsout_t = dram.tile([BAGS, NW], bf16)

        # --- constants; weights then tabt pieces at the head of the sync
        # queue (earliest issuer; first ~10us is fixed barrier preamble),
        # rhsc + epilogue constants on gpsimd ---
        w1t_sb = const.tile([128, 2, H], f16)
        nc.sync.dma_start(w1t_sb[:, 0, :], w1t_d.ap()[0:128, :])
        nc.sync.dma_start(w1t_sb[:, 1, :], w1t_d.ap()[128:256, :])
        w2c_sb = const.tile([H, 1], f16)
        nc.sync.dma_start(w2c_sb[:], w2c_d.ap())
        b1_sb = const.tile([H, 1], f32)
        nc.sync.dma_start(b1_sb[:], b1_d.ap())
        tabt_sb = const.tile([128, 2, VSL], f16)
        rhsc_sb = const.tile([128, NCH, NW], f16)
        for s in range(VSL // NSL):
            ssl = slice(s * NSL, (s + 1) * NSL)
            nc.sync.dma_start(
                tabt_sb[:, :, ssl],
                tabt_d.ap()[:, :].rearrange("p (a b) -> p a b", a=2)[:, :, ssl])
        CPS = NCH // (VSL // NSL)  # rhs chunks per MLP slice
        for s in range(VSL // NSL):
            nc.gpsimd.dma_start(
                rhsc_sb[:, s * CPS:(s + 1) * CPS, :].rearrange(
                    "p a b -> p (a b)"),
                rhsc_d.ap()[:, s * CPS * NW:(s + 1) * CPS * NW])
        epool_sb = const.tile([128, 16, 32], f16)
        nc.gpsimd.dma_start(epool_sb[:].rearrange("p a b -> p (a b)"),
                            epool_d.ap())
        egidx_sb = const.tile([128, ECODES // 16], i16)
        nc.gpsimd.dma_start(egidx_sb[:], egidx_d.ap())

        g_sb = const.tile([128, NCH], f32)
        tg_sb = const.tile([128, NCH, NW], f16)
        g_ps = gpp.tile([128, NCH], f32)
        egat = const.tile([128, ECODES // 128, D], f16)

        # --- score MLP over the vocab slice, then per-chunk rhs scaling ---
        for s in range(VSL // NSL):
            ssl = slice(s * NSL, (s + 1) * NSL)
            ph = php.tile([128, NSL], f32)
            nc.tensor.matmul(ph[:], w1t_sb[:, 0, :], tabt_sb[:, 0, ssl],
                             start=True, stop=False)
            nc.tensor.matmul(ph[:], w1t_sb[:, 1, :], tabt_sb[:, 1, ssl],
                             start=False, stop=True)
            h1 = hp.tile([128, NSL], f16)
            nc.scalar.activation(h1[:], ph[:],
                                 mybir.ActivationFunctionType.Tanh,
                                 bias=b1_sb[:], scale=1.0)
            for k in range(NSL // 128):
                j = s * (NSL // 128) + k
                nc.tensor.matmul(g_ps[:, j:j + 1],
                                 h1[:, k * 128:(k + 1) * 128], w2c_sb[:],
                                 start=True, stop=True)
            jsl = slice(s * (NSL // 128), (s + 1) * (NSL // 128))
            nc.scalar.activation(g_sb[:, jsl], g_ps[:, jsl],
                                 mybir.ActivationFunctionType.Exp)
            for k in range(NSL // 128):
                j = s * (NSL // 128) + k
                nc.vector.tensor_scalar(tg_sb[:, j, :], rhsc_sb[:, j, :],
                                        g_sb[:, j:j + 1], None,
                                        mybir.AluOpType.mult)

        # --- main loop: K-contiguous per 128-bag tile ---
        for t in range(NT):
            ct = cntp.tile([128, NCH, 128], f8)
            nc.sync.dma_start(ct[:].rearrange("p a b -> p (a b)"),
                              cnt_d.ap()[:, t * NCH * 128:(t + 1) * NCH * 128])
            if t == 3:
                # len-0 epilogue gather rides the stream mid-loop
                for k in range(ECODES // 1024):
                    nc.gpsimd.dma_gather(
                        egat[:, k * 8:(k + 1) * 8, :], table_d.ap(),
                        egidx_sb[:, k * 64:(k + 1) * 64], 1024, 1024, D)
            ps = psp.tile([128, NW], f32)
            for j in range(NCH):
                nc.tensor.matmul(ps[:], ct[:, j, :], tg_sb[:, j, :],
                                 start=(j == 0), stop=(j == NCH - 1))
            ob = obp.tile([128, NW], bf16)
            nc.vector.tensor_copy(ob[:], ps[:])
            nc.sync.dma_start(part_t[t * 128:(t + 1) * 128, :], ob[:])
        nc.gpsimd.collective_compute(
            "ReduceScatter", mybir.AluOpType.add, groups,
            ins=[part_t[:, :]], outs=[rsout_t[:, :]])

        # --- len-0 epilogue: mean over 64 codes per slot, no collective ---
        eps = epp.tile([ESLOT, D], f32)
        for gb in range(16):
            nc.tensor.matmul(eps[:], epool_sb[:, gb, :], egat[:, gb, :],
                             start=(gb == 0), stop=(gb == 15))
        eout = normp.tile([ESLOT, D], f32, tag="eout")
        nc.vector.tensor_copy(eout[:], eps[:])
        nc.gpsimd.dma_start(out_d.ap()[BAGS:ORT, :], eout[:])

        # --- readback, normalize, store (gpsimd queue; sem-gated at tail) ---
        parts = [(0, 128), (128, 128), (256, 128), (384, 16)]
        its, rzs = [], []
        for n, (off, m) in enumerate(parts):
            it = normp.tile([128, NW], bf16, tag=f"it{n}")
            nc.gpsimd.dma_start(it[0:m, :], rsout_t[off:off + m, :])
            its.append(it)
        for n, (off, m) in enumerate(parts):
            zc = normp.tile([128, 1], f32, tag=f"zc{n}")
            # len-0 bag rows have Z == 0 exactly; host overwrites them
            nc.vector.tensor_scalar_max(zc[0:m], its[n][0:m, D:D + 1], 1e-20)
            rz = normp.tile([128, 1], f32, tag=f"rz{n}")
            nc.vector.reciprocal(rz[0:m], zc[0:m])
            rzs.append(rz)
        for n, (off, m) in enumerate(parts):
            osb = normp.tile([128, D], f32, tag=f"osb{n}")
            nc.vector.tensor_scalar(osb[0:m, :], its[n][0:m, 0:D], rzs[n][0:m],
                                    None, mybir.AluOpType.mult)
            nc.gpsimd.dma_start(out_d.ap()[off:off + m, :], osb[0:m, :])

    nc.compile()
    return nc


def _wrap16(idx_flat):
    n = idx_flat.shape[0]
    return idx_flat.reshape(n // 16, 16).T.copy()


def _prep_shared(embed_table, W1, b1, W2):
    """Per-core-sliceable views of the table + tiny MLP weights."""
    t16 = embed_table.astype(np.float16)                      # [20000, 256]
    tabt = np.zeros((D, VP), np.float16)
    tabt[:, :NUM_CODE] = t16.T
    rhsc = np.zeros((VP, NW), np.float16)
    rhsc[:NUM_CODE, :D] = t16
    rhsc[:NUM_CODE, D] = 1.0
    w1t = np.ascontiguousarray(W1.astype(np.float16).T)       # [256, 128]
    w2c = np.ascontiguousarray(W2.astype(np.float16).reshape(H, 1))
    b1c = np.ascontiguousarray(b1.astype(np.float32).reshape(H, 1))
    epool = np.zeros((128, 16, 32), np.float16)
    for g in range(16):
        epool[0:64, g, 2 * g] = 1.0 / C
        epool[64:128, g, 2 * g + 1] = 1.0 / C
    epool = epool.reshape(128, 16 * 32)
    return dict(table=t16, tabt=tabt, rhsc=rhsc, w1t=w1t, w2c=w2c, b1=b1c,
                epool=epool)


def build_in_maps(input_code, length_code, shared):
    import ml_dtypes

    codes = input_code.reshape(GBAGS, C).astype(np.int64)
    lens = length_code.reshape(GBAGS).astype(np.int64)

    # global column order: plain core-major (col = core*400 + i)
    cnt = np.zeros((VP, GBAGS), np.float32)
    valid = np.arange(C)[None, :] < lens[:, None]
    bb, cc = np.nonzero(valid)
    np.add.at(cnt, (codes[bb, cc], bb), 1.0)

    in_maps = []
    len0_lists = []
    for core in range(NCORES):
        vs = slice(core * VSL, (core + 1) * VSL)
        cntl = np.ascontiguousarray(
            cnt[vs].reshape(NCH, 128, NT, 128).transpose(1, 2, 0, 3)
        ).astype(ml_dtypes.float8_e4m3).reshape(128, NT * NCH * 128)
        tabtc = np.ascontiguousarray(
            shared["tabt"][:, vs].reshape(2, 128, VSL).transpose(1, 0, 2)
        ).reshape(128, 2 * VSL)
        rhscc = np.ascontiguousarray(
            shared["rhsc"][vs].reshape(NCH, 128, NW).transpose(1, 0, 2)
        ).reshape(128, NCH * NW)
        len0 = np.nonzero(lens[core * BAGS:(core + 1) * BAGS] == 0)[0][:ESLOT]
        ecodes = np.zeros(ECODES, np.int16)
        for s, b in enumerate(len0):
            ecodes[s * C:(s + 1) * C] = codes[core * BAGS + b]
        egidx = np.tile(_wrap16(ecodes), (8, 1))               # [128, 128]
        len0_lists.append(len0)
        in_maps.append(dict(tabt=tabtc, rhsc=rhscc, cnt=cntl,
                            w1t=shared["w1t"], w2c=shared["w2c"],
                            b1=shared["b1"], table=shared["table"],
                            epool=shared["epool"], egidx=egidx))
    return in_maps, len0_lists


def kernel(input_code, length_code, embed_table, W1, b1, W2, b2):
    from concourse.bass_utils import run_bass_kernel_spmd

    if "nc" not in _cache:
        _cache["nc"] = _build_program()
    nc = _cache["nc"]

    shared = _prep_shared(np.asarray(embed_table), np.asarray(W1),
                          np.asarray(b1), np.asarray(W2))
    in_maps, len0_lists = build_in_maps(np.asarray(input_code),
                                        np.asarray(length_code), shared)
    res = run_bass_kernel_spmd(nc, in_maps, core_ids=list(range(NCORES)))
    outs = []
    for c in range(NCORES):
        full = res.results[c]["out"]
        o = full[:BAGS].copy()
        for s, b in enumerate(len0_lists[c]):
            o[b] = full[BAGS + s]
        outs.append(o.reshape(BPC, V, D))
    return np.concatenate(outs, axis=0)
